# revision 9
# baseline (speedup 1.0000x reference)
"""Multi-head attention kernel for 8 TRN2 NeuronCores.

Reference: out = einsum('dha,blha->bld', O, softmax(q k^T) v) with
q/k/v = einsum('dha,bld->blha', W, x);  B=4, L=2048, D=1024, H=16, A=64.

Sharding: core c handles batch b = c//2 and head-group hg = c%2 (8 heads).
Each core computes a partial output [L, D] summed over its 8 heads; the host
adds the two head-group partials per batch.

Per-core layout (all "T" = transposed so contractions sit on SBUF partitions):
  phase 1: qT/kT/vT = W^T @ xT via fp32r matmuls (x streamed fp32r from DRAM,
           head pairs packed to M=128, weights stationary reused across two
           lq tiles); v then PE-transposed to natural [Lk, A] bf16 with a
           ones column (softmax denominators come free in the ctx matmul).
  phase 2: per head pair, interleaved even/odd so the K=64 scores matmuls
           land in disjoint PE row groups (tile_position row packing):
           scoresT[lk,lq] = kT^T qT (fp32r); exp on ACT psum->sbuf bf16
           (no max subtraction: |scores| < ~60 so fp32 exp is safe);
           ctx_aug[65,lq] accumulates v_aug^T @ expT over 16 lk chunks;
           normalize = reciprocal_approx_fast on the denominator row +
           DRAM-bounce partition broadcast + DVE multiply -> ctxT pair tile
           [128, lq] (odd head placed via SBUF->SBUF DMA); output projection
           K=128 over pair tiles, heads summed in PSUM -> DMA fp32 out.
"""

import sys

sys.path.insert(0, "/opt/trn_rl_repo")

from contextlib import ExitStack

import numpy as np
import ml_dtypes

import concourse.bass as bass  # noqa: F401
import concourse.tile as tile
from concourse import bacc, mybir
from concourse.bass_utils import run_bass_kernel_spmd
from concourse.masks import make_identity

B, L, D, H, A = 4, 2048, 1024, 16, 64
HC = 8          # heads per core
NP = HC // 2    # head pairs per core
DC = D // 128   # d chunks
LC = L // 128   # l chunks

f32 = mybir.dt.float32
bf16 = mybir.dt.bfloat16
f32r = mybir.dt.float32r
ExpF = mybir.ActivationFunctionType.Exp


def build_graph():
    nc = bacc.Bacc("TRN2", target_bir_lowering=False, debug=False, num_devices=8)
    xqT_e = nc.dram_tensor("xqT", [D, L], f32r, kind="ExternalInput").ap()
    xkvT_e = nc.dram_tensor("xkvT", [D, L], f32r, kind="ExternalInput").ap()
    Qw_e = nc.dram_tensor("Qw", [D, HC * A], f32r, kind="ExternalInput").ap()
    Kw_e = nc.dram_tensor("Kw", [D, HC * A], f32r, kind="ExternalInput").ap()
    Vw_e = nc.dram_tensor("Vw", [D, HC * A], f32r, kind="ExternalInput").ap()
    OwT_e = nc.dram_tensor("OwT", [HC * A, D], bf16, kind="ExternalInput").ap()
    out_e = nc.dram_tensor("out", [L, D], f32, kind="ExternalOutput").ap()

    with tile.TileContext(nc) as tc, ExitStack() as ctx:
        pers = ctx.enter_context(tc.tile_pool(name="pers", bufs=1))
        qT = [pers.tile([128, L], f32r, tag=f"qT{p}", name=f"qT{p}") for p in range(NP)]
        kT = [pers.tile([128, L], f32r, tag=f"kT{p}", name=f"kT{p}") for p in range(NP)]
        # v_aug[h]: [lk chunk part, chunk, 0:64 v | 64 ones | 65 pad]
        vaug = [
            pers.tile([128, LC, 66], bf16, tag=f"vaug{h}", name=f"vaug{h}")
            for h in range(HC)
        ]
        ident = pers.tile([128, 128], bf16, tag="ident", name="ident")
        make_identity(nc, ident[:])
        for h in range(HC):
            nc.vector.memset(vaug[h][:, :, 64:65], 1.0)

        # ---------------- phase 1: projections ----------------
        with tc.tile_pool(name="wp", bufs=1) as wp, \
             tc.tile_pool(name="xin", bufs=4) as xp, \
             tc.tile_pool(name="vtp", bufs=1) as vtp, \
             tc.tile_pool(name="pp1", bufs=8, space="PSUM") as pp1:
            Qc = [wp.tile([128, HC * A], f32r, tag=f"Qc{d}", name=f"Qc{d}") for d in range(DC)]
            Kc = [wp.tile([128, HC * A], f32r, tag=f"Kc{d}", name=f"Kc{d}") for d in range(DC)]
            Vc = [wp.tile([128, HC * A], f32r, tag=f"Vc{d}", name=f"Vc{d}") for d in range(DC)]
            for d in range(DC):
                nc.sync.dma_start(out=Qc[d][:], in_=Qw_e[d * 128:(d + 1) * 128, :])
                nc.sync.dma_start(out=Kc[d][:], in_=Kw_e[d * 128:(d + 1) * 128, :])
                nc.sync.dma_start(out=Vc[d][:], in_=Vw_e[d * 128:(d + 1) * 128, :])
            vT = [vtp.tile([128, L], bf16, tag=f"vT{p}", name=f"vT{p}") for p in range(NP)]

            # one pass per projection; weights stationary reused across the
            # two 512-wide lq tiles of each half, 8 psum accumulators live
            def proj_pass(which, x_e, Wc, emit_out):
                for lqh in range(2):
                    ps = [
                        [
                            pp1.tile([128, 512], f32, tag="qk", bufs=8,
                                     name=f"ps_{which}_{lqh}_{p}_{j}")
                            for j in range(2)
                        ]
                        for p in range(NP)
                    ]
                    for d in range(DC):
                        xt = xp.tile([128, 1024], f32r, tag="x", bufs=4,
                                     name=f"x_{which}_{lqh}_{d}")
                        for j in range(2):
                            lo = lqh * 1024 + j * 512
                            nc.sync.dma_start(
                                out=xt[:, j * 512:(j + 1) * 512],
                                in_=x_e[d * 128:(d + 1) * 128, lo:lo + 512])
                        for p in range(NP):
                            for j in range(2):
                                nc.tensor.matmul(
                                    ps[p][j][:],
                                    lhsT=Wc[d][:, p * 128:(p + 1) * 128],
                                    rhs=xt[:, j * 512:(j + 1) * 512],
                                    start=(d == 0), stop=(d == DC - 1))
                    for p in range(NP):
                        for j in range(2):
                            emit_out(p, lqh * 2 + j, ps[p][j])

            proj_pass("q", xqT_e, Qc,
                      lambda p, lq, pst: nc.scalar.copy(
                          qT[p][:, lq * 512:(lq + 1) * 512], pst[:]))
            proj_pass("k", xkvT_e, Kc,
                      lambda p, lq, pst: nc.scalar.copy(
                          kT[p][:, lq * 512:(lq + 1) * 512], pst[:]))
            proj_pass("v", xkvT_e, Vc,
                      lambda p, lq, pst: nc.vector.tensor_copy(
                          vT[p][:, lq * 512:(lq + 1) * 512], pst[:]))

            # v transposes: vT [2h*a, lk] -> v natural [lk, a] per head
            for p in range(NP):
                for c in range(LC):
                    pt = pp1.tile([128, 128], bf16, tag="qk", bufs=8, name=f"pt{p}_{c}")
                    nc.tensor.transpose(pt[:], vT[p][:, c * 128:(c + 1) * 128], ident[:])
                    nc.vector.tensor_copy(vaug[2 * p][:, c, 0:64], pt[:, 0:64])
                    nc.vector.tensor_copy(vaug[2 * p + 1][:, c, 0:64], pt[:, 64:128])

        # ---------------- phase 2: attention + output projection ----------------
        with tc.tile_pool(name="owp", bufs=1) as owp, \
             tc.tile_pool(name="p2p", bufs=1) as p2p, \
             tc.tile_pool(name="drp", bufs=4, space="DRAM") as drp, \
             tc.tile_pool(name="expp", bufs=8) as ep, \
             tc.tile_pool(name="pp2", bufs=1, space="PSUM") as pp2:
            # O weights pair-stacked: chunk c rows = (head 2c | head 2c+1) x a
            ow = [owp.tile([128, D], bf16, tag=f"ow{c}", name=f"ow{c}") for c in range(NP)]
            for c in range(NP):
                nc.sync.dma_start(out=ow[c][:], in_=OwT_e[c * 128:(c + 1) * 128, :])

            for strip in range(2):
                ctxp = []
                for p in range(NP):
                    ctp = p2p.tile([128, 1024], bf16, tag="ctxT", bufs=6,
                                   name=f"ctp{strip}_{p}")
                    pcs = [
                        [
                            pp2.tile([65, 512], f32, tag="c", bufs=4,
                                     name=f"pc{strip}_{p}_{h2}_{s}")
                            for s in range(2)
                        ]
                        for h2 in range(2)
                    ]
                    for c in range(LC):
                        sts = [
                            pp2.tile([128, 1024], f32, tag="s", bufs=2,
                                     name=f"st{strip}_{p}_{h2}_{c}")
                            for h2 in range(2)
                        ]
                        # interleave even/odd head matmuls: adjacent pairs sit
                        # in disjoint PE row groups (bases 0 and 64) and the
                        # hardware runs them concurrently
                        for sub in range(2):
                            lo = strip * 1024 + sub * 512
                            for h2 in range(2):
                                base = 64 * h2
                                nc.tensor.matmul(
                                    sts[h2][:, sub * 512:(sub + 1) * 512],
                                    lhsT=kT[p][base:base + 64, c * 128:(c + 1) * 128],
                                    rhs=qT[p][base:base + 64, lo:lo + 512],
                                    start=True, stop=True)
                        for h2 in range(2):
                            et = ep.tile([128, 1024], bf16, tag="exp",
                                         name=f"et{strip}_{p}_{h2}_{c}")
                            nc.scalar.activation(et[:], sts[h2][:], ExpF)
                            for sub in range(2):
                                nc.tensor.matmul(
                                    pcs[h2][sub][:],
                                    lhsT=vaug[2 * p + h2][:, c, 0:65],
                                    rhs=et[:, sub * 512:(sub + 1) * 512],
                                    start=(c == 0), stop=(c == LC - 1))
                    for h2 in range(2):
                        cto = None
                        if h2 == 1:
                            cto = p2p.tile([64, 1024], bf16, tag="cto", bufs=3,
                                           name=f"cto{strip}_{p}")
                        for sub in range(2):
                            rt = p2p.tile([65, 512], f32, tag="recip", bufs=2,
                                          name=f"rt{strip}_{p}_{h2}_{sub}")
                            nc.vector.reciprocal(
                                rt[64:65, :], pcs[h2][sub][64:65, :])
                            dr = drp.tile([1, 512], f32, tag="dr", bufs=4,
                                          name=f"dr{strip}_{p}_{h2}_{sub}")
                            nc.sync.dma_start(out=dr[:], in_=rt[64:65, :])
                            pbs = p2p.tile([64, 512], f32, tag="bcast", bufs=2,
                                           name=f"pbs{strip}_{p}_{h2}_{sub}")
                            dr_bcast = bass.AP(
                                tensor=dr[:].tensor,
                                offset=dr[:].offset,
                                ap=[[0, 64], [1, 512]],
                            )
                            nc.sync.dma_start(out=pbs[:], in_=dr_bcast)
                            dst = (ctp[0:64, sub * 512:(sub + 1) * 512]
                                   if h2 == 0 else
                                   cto[:, sub * 512:(sub + 1) * 512])
                            nc.vector.tensor_mul(
                                dst, pcs[h2][sub][0:64, :], pbs[:])
                        if h2 == 1:
                            # odd head into pair-tile partitions 64..127
                            nc.sync.dma_start(out=ctp[64:128, :], in_=cto[:])
                    ctxp.append(ctp)

                # output projection for this strip: K=128 per pair, pairs
                # accumulated in PSUM
                for lqs in range(8):
                    for dt_ in range(2):
                        po = pp2.tile([128, 512], f32, tag="c", bufs=4,
                                      name=f"po{strip}_{lqs}_{dt_}")
                        for p in range(NP):
                            nc.tensor.matmul(
                                po[:],
                                lhsT=ctxp[p][:, lqs * 128:(lqs + 1) * 128],
                                rhs=ow[p][:, dt_ * 512:(dt_ + 1) * 512],
                                start=(p == 0), stop=(p == NP - 1))
                        ost = p2p.tile([128, 512], f32, tag="ost", bufs=3,
                                       name=f"ost{strip}_{lqs}_{dt_}")
                        nc.vector.tensor_copy(ost[:], po[:])
                        row = strip * 1024 + lqs * 128
                        nc.sync.dma_start(
                            out=out_e[row:row + 128, dt_ * 512:(dt_ + 1) * 512],
                            in_=ost[:])

    nc.compile()
    return nc


_NC = None


def _get_nc():
    global _NC
    if _NC is None:
        _NC = build_graph()
    return _NC


# test harness can override, e.g. {"trace": True}
RUN_KWARGS: dict = {}
LAST_RESULTS = None


def make_in_maps(xq, xkv, Q, K, V, O):
    xq = np.asarray(xq, np.float32)
    xkv = np.asarray(xkv, np.float32)
    Q = np.asarray(Q, np.float32)
    K = np.asarray(K, np.float32)
    V = np.asarray(V, np.float32)
    O = np.asarray(O, np.float32)
    in_maps = []
    for core in range(8):
        b, hg = divmod(core, 2)
        hs = slice(hg * HC, (hg + 1) * HC)
        in_maps.append({
            "xqT": np.ascontiguousarray(xq[b].T),
            "xkvT": np.ascontiguousarray(xkv[b].T),
            "Qw": np.ascontiguousarray(Q[:, hs, :].reshape(D, HC * A)),
            "Kw": np.ascontiguousarray(K[:, hs, :].reshape(D, HC * A)),
            "Vw": np.ascontiguousarray(V[:, hs, :].reshape(D, HC * A)),
            "OwT": np.ascontiguousarray(
                O[:, hs, :].reshape(D, HC * A).T).astype(ml_dtypes.bfloat16),
        })
    return in_maps


def kernel(xq, xkv, Q, K, V, O):
    global LAST_RESULTS
    nc = _get_nc()
    in_maps = make_in_maps(xq, xkv, Q, K, V, O)
    res = run_bass_kernel_spmd(nc, in_maps, core_ids=list(range(8)), **RUN_KWARGS)
    LAST_RESULTS = res
    outs = [np.asarray(res.results[c]["out"], np.float32) for c in range(8)]
    return np.stack([outs[2 * b] + outs[2 * b + 1] for b in range(B)], axis=0)


# revision 10
# speedup vs baseline: 1.0834x; 1.0834x over previous
"""Multi-head attention kernel for 8 TRN2 NeuronCores.

Reference: out = einsum('dha,blha->bld', O, softmax(q k^T) v) with
q/k/v = einsum('dha,bld->blha', W, x);  B=4, L=2048, D=1024, H=16, A=64.

Sharding: core c handles batch b = c//2 and head-group hg = c%2 (8 heads).
Each core computes a partial output [L, D] summed over its 8 heads; the host
adds the two head-group partials per batch.

Per-core layout (all "T" = transposed so contractions sit on SBUF partitions):
  phase 1: qT/kT/vT = W^T @ xT via fp32r matmuls (x streamed fp32r from DRAM,
           head pairs packed to M=128, weights stationary reused across two
           lq tiles); v then PE-transposed to natural [Lk, A] bf16 with a
           ones column (softmax denominators come free in the ctx matmul).
  phase 2: per head pair, interleaved even/odd so the K=64 scores matmuls
           land in disjoint PE row groups (tile_position row packing):
           scoresT[lk,lq] = kT^T qT (fp32r); exp on ACT psum->sbuf bf16
           (no max subtraction: |scores| < ~60 so fp32 exp is safe);
           ctx_aug[65,lq] accumulates v_aug^T @ expT over 16 lk chunks;
           normalize = reciprocal_approx_fast on the denominator row +
           DRAM-bounce partition broadcast + DVE multiply -> ctxT pair tile
           [128, lq] (odd head placed via SBUF->SBUF DMA); output projection
           K=128 over pair tiles, heads summed in PSUM -> DMA fp32 out.
"""

import sys

sys.path.insert(0, "/opt/trn_rl_repo")

from contextlib import ExitStack

import numpy as np
import ml_dtypes

import concourse.bass as bass  # noqa: F401
import concourse.tile as tile
from concourse import bacc, mybir
from concourse.bass_utils import run_bass_kernel_spmd
from concourse.masks import make_identity

B, L, D, H, A = 4, 2048, 1024, 16, 64
HC = 8          # heads per core
NP = HC // 2    # head pairs per core
DC = D // 128   # d chunks
LC = L // 128   # l chunks

f32 = mybir.dt.float32
bf16 = mybir.dt.bfloat16
f32r = mybir.dt.float32r
ExpF = mybir.ActivationFunctionType.Exp


def build_graph():
    nc = bacc.Bacc("TRN2", target_bir_lowering=False, debug=False, num_devices=8)
    xqT_e = nc.dram_tensor("xqT", [D, L], f32r, kind="ExternalInput").ap()
    xkvT_e = nc.dram_tensor("xkvT", [D, L], f32r, kind="ExternalInput").ap()
    Qw_e = nc.dram_tensor("Qw", [D, HC * A], f32r, kind="ExternalInput").ap()
    Kw_e = nc.dram_tensor("Kw", [D, HC * A], f32r, kind="ExternalInput").ap()
    Vw_e = nc.dram_tensor("Vw", [D, HC * A], f32r, kind="ExternalInput").ap()
    OwT_e = nc.dram_tensor("OwT", [HC * A, D], bf16, kind="ExternalInput").ap()
    out_e = nc.dram_tensor("out", [L, D], f32, kind="ExternalOutput").ap()

    with tile.TileContext(nc) as tc, ExitStack() as ctx:
        pers = ctx.enter_context(tc.tile_pool(name="pers", bufs=1))
        qT = [pers.tile([128, L], bf16, tag=f"qT{p}", name=f"qT{p}") for p in range(NP)]
        kT = [pers.tile([128, L], bf16, tag=f"kT{p}", name=f"kT{p}") for p in range(NP)]
        # v_aug[h]: [lk chunk part, chunk, 0:64 v | 64 ones | 65 pad]
        vaug = [
            pers.tile([128, LC, 66], bf16, tag=f"vaug{h}", name=f"vaug{h}")
            for h in range(HC)
        ]
        ident = pers.tile([128, 128], bf16, tag="ident", name="ident")
        make_identity(nc, ident[:])
        for h in range(HC):
            nc.vector.memset(vaug[h][:, :, 64:65], 1.0)

        # ---------------- phase 1: projections ----------------
        with tc.tile_pool(name="wp", bufs=1) as wp, \
             tc.tile_pool(name="xin", bufs=4) as xp, \
             tc.tile_pool(name="vtp", bufs=1) as vtp, \
             tc.tile_pool(name="pp1", bufs=8, space="PSUM") as pp1:
            Qc = [wp.tile([128, HC * A], f32r, tag=f"Qc{d}", name=f"Qc{d}") for d in range(DC)]
            Kc = [wp.tile([128, HC * A], f32r, tag=f"Kc{d}", name=f"Kc{d}") for d in range(DC)]
            Vc = [wp.tile([128, HC * A], f32r, tag=f"Vc{d}", name=f"Vc{d}") for d in range(DC)]
            for d in range(DC):
                nc.sync.dma_start(out=Qc[d][:], in_=Qw_e[d * 128:(d + 1) * 128, :])
                nc.sync.dma_start(out=Kc[d][:], in_=Kw_e[d * 128:(d + 1) * 128, :])
                nc.sync.dma_start(out=Vc[d][:], in_=Vw_e[d * 128:(d + 1) * 128, :])
            vT = [vtp.tile([128, L], bf16, tag=f"vT{p}", name=f"vT{p}") for p in range(NP)]

            # one pass per projection; weights stationary reused across the
            # two 512-wide lq tiles of each half, 8 psum accumulators live
            def proj_pass(which, x_e, Wc, emit_out):
                for lqh in range(2):
                    ps = [
                        [
                            pp1.tile([128, 512], f32, tag="qk", bufs=8,
                                     name=f"ps_{which}_{lqh}_{p}_{j}")
                            for j in range(2)
                        ]
                        for p in range(NP)
                    ]
                    for d in range(DC):
                        xt = xp.tile([128, 1024], f32r, tag="x", bufs=4,
                                     name=f"x_{which}_{lqh}_{d}")
                        for j in range(2):
                            lo = lqh * 1024 + j * 512
                            nc.sync.dma_start(
                                out=xt[:, j * 512:(j + 1) * 512],
                                in_=x_e[d * 128:(d + 1) * 128, lo:lo + 512])
                        for p in range(NP):
                            for j in range(2):
                                nc.tensor.matmul(
                                    ps[p][j][:],
                                    lhsT=Wc[d][:, p * 128:(p + 1) * 128],
                                    rhs=xt[:, j * 512:(j + 1) * 512],
                                    start=(d == 0), stop=(d == DC - 1))
                    for p in range(NP):
                        for j in range(2):
                            emit_out(p, lqh * 2 + j, ps[p][j])

            proj_pass("q", xqT_e, Qc,
                      lambda p, lq, pst: nc.vector.tensor_copy(
                          qT[p][:, lq * 512:(lq + 1) * 512], pst[:]))
            proj_pass("k", xkvT_e, Kc,
                      lambda p, lq, pst: nc.vector.tensor_copy(
                          kT[p][:, lq * 512:(lq + 1) * 512], pst[:]))
            proj_pass("v", xkvT_e, Vc,
                      lambda p, lq, pst: nc.vector.tensor_copy(
                          vT[p][:, lq * 512:(lq + 1) * 512], pst[:]))

            # v transposes: vT [2h*a, lk] -> v natural [lk, a] per head
            for p in range(NP):
                for c in range(LC):
                    pt = pp1.tile([128, 128], bf16, tag="qk", bufs=8, name=f"pt{p}_{c}")
                    nc.tensor.transpose(pt[:], vT[p][:, c * 128:(c + 1) * 128], ident[:])
                    nc.vector.tensor_copy(vaug[2 * p][:, c, 0:64], pt[:, 0:64])
                    nc.vector.tensor_copy(vaug[2 * p + 1][:, c, 0:64], pt[:, 64:128])

        # ---------------- phase 2: attention + output projection ----------------
        with tc.tile_pool(name="owp", bufs=1) as owp, \
             tc.tile_pool(name="p2p", bufs=1) as p2p, \
             tc.tile_pool(name="drp", bufs=4, space="DRAM") as drp, \
             tc.tile_pool(name="expp", bufs=8) as ep, \
             tc.tile_pool(name="pp2", bufs=1, space="PSUM") as pp2:
            # O weights pair-stacked: chunk c rows = (head 2c | head 2c+1) x a
            ow = [owp.tile([128, D], bf16, tag=f"ow{c}", name=f"ow{c}") for c in range(NP)]
            for c in range(NP):
                nc.sync.dma_start(out=ow[c][:], in_=OwT_e[c * 128:(c + 1) * 128, :])

            for strip in range(2):
                ctxp = []
                for p in range(NP):
                    ctp = p2p.tile([128, 1024], bf16, tag="ctxT", bufs=6,
                                   name=f"ctp{strip}_{p}")
                    pcs = [
                        [
                            pp2.tile([65, 512], f32, tag="c", bufs=4,
                                     name=f"pc{strip}_{p}_{h2}_{s}")
                            for s in range(2)
                        ]
                        for h2 in range(2)
                    ]
                    for c in range(LC):
                        sts = [
                            pp2.tile([128, 1024], f32, tag="s", bufs=2,
                                     name=f"st{strip}_{p}_{h2}_{c}")
                            for h2 in range(2)
                        ]
                        # interleave even/odd head matmuls: adjacent pairs sit
                        # in disjoint PE row groups (bases 0 and 64) and the
                        # hardware runs them concurrently
                        for sub in range(2):
                            lo = strip * 1024 + sub * 512
                            for h2 in range(2):
                                base = 64 * h2
                                nc.tensor.matmul(
                                    sts[h2][:, sub * 512:(sub + 1) * 512],
                                    lhsT=kT[p][base:base + 64, c * 128:(c + 1) * 128],
                                    rhs=qT[p][base:base + 64, lo:lo + 512],
                                    start=True, stop=True)
                        for h2 in range(2):
                            et = ep.tile([128, 1024], bf16, tag="exp",
                                         name=f"et{strip}_{p}_{h2}_{c}")
                            nc.scalar.activation(et[:], sts[h2][:], ExpF)
                            for sub in range(2):
                                nc.tensor.matmul(
                                    pcs[h2][sub][:],
                                    lhsT=vaug[2 * p + h2][:, c, 0:65],
                                    rhs=et[:, sub * 512:(sub + 1) * 512],
                                    start=(c == 0), stop=(c == LC - 1))
                    for h2 in range(2):
                        cto = None
                        if h2 == 1:
                            cto = p2p.tile([64, 1024], bf16, tag="cto", bufs=3,
                                           name=f"cto{strip}_{p}")
                        for sub in range(2):
                            rt = p2p.tile([65, 512], f32, tag="recip", bufs=2,
                                          name=f"rt{strip}_{p}_{h2}_{sub}")
                            nc.vector.reciprocal(
                                rt[64:65, :], pcs[h2][sub][64:65, :])
                            dr = drp.tile([1, 512], f32, tag="dr", bufs=4,
                                          name=f"dr{strip}_{p}_{h2}_{sub}")
                            nc.sync.dma_start(out=dr[:], in_=rt[64:65, :])
                            pbs = p2p.tile([64, 512], f32, tag="bcast", bufs=2,
                                           name=f"pbs{strip}_{p}_{h2}_{sub}")
                            dr_bcast = bass.AP(
                                tensor=dr[:].tensor,
                                offset=dr[:].offset,
                                ap=[[0, 64], [1, 512]],
                            )
                            nc.sync.dma_start(out=pbs[:], in_=dr_bcast)
                            dst = (ctp[0:64, sub * 512:(sub + 1) * 512]
                                   if h2 == 0 else
                                   cto[:, sub * 512:(sub + 1) * 512])
                            nc.vector.tensor_mul(
                                dst, pcs[h2][sub][0:64, :], pbs[:])
                        if h2 == 1:
                            # odd head into pair-tile partitions 64..127
                            nc.sync.dma_start(out=ctp[64:128, :], in_=cto[:])
                    ctxp.append(ctp)

                # output projection for this strip: K=128 per pair, pairs
                # accumulated in PSUM
                for lqs in range(8):
                    for dt_ in range(2):
                        po = pp2.tile([128, 512], f32, tag="c", bufs=4,
                                      name=f"po{strip}_{lqs}_{dt_}")
                        for p in range(NP):
                            nc.tensor.matmul(
                                po[:],
                                lhsT=ctxp[p][:, lqs * 128:(lqs + 1) * 128],
                                rhs=ow[p][:, dt_ * 512:(dt_ + 1) * 512],
                                start=(p == 0), stop=(p == NP - 1))
                        ost = p2p.tile([128, 512], f32, tag="ost", bufs=3,
                                       name=f"ost{strip}_{lqs}_{dt_}")
                        nc.vector.tensor_copy(ost[:], po[:])
                        row = strip * 1024 + lqs * 128
                        nc.sync.dma_start(
                            out=out_e[row:row + 128, dt_ * 512:(dt_ + 1) * 512],
                            in_=ost[:])

    nc.compile()
    return nc


_NC = None


def _get_nc():
    global _NC
    if _NC is None:
        _NC = build_graph()
    return _NC


# test harness can override, e.g. {"trace": True}
RUN_KWARGS: dict = {}
LAST_RESULTS = None


def make_in_maps(xq, xkv, Q, K, V, O):
    xq = np.asarray(xq, np.float32)
    xkv = np.asarray(xkv, np.float32)
    Q = np.asarray(Q, np.float32)
    K = np.asarray(K, np.float32)
    V = np.asarray(V, np.float32)
    O = np.asarray(O, np.float32)
    in_maps = []
    for core in range(8):
        b, hg = divmod(core, 2)
        hs = slice(hg * HC, (hg + 1) * HC)
        in_maps.append({
            "xqT": np.ascontiguousarray(xq[b].T),
            "xkvT": np.ascontiguousarray(xkv[b].T),
            "Qw": np.ascontiguousarray(Q[:, hs, :].reshape(D, HC * A)),
            "Kw": np.ascontiguousarray(K[:, hs, :].reshape(D, HC * A)),
            "Vw": np.ascontiguousarray(V[:, hs, :].reshape(D, HC * A)),
            "OwT": np.ascontiguousarray(
                O[:, hs, :].reshape(D, HC * A).T).astype(ml_dtypes.bfloat16),
        })
    return in_maps


def kernel(xq, xkv, Q, K, V, O):
    global LAST_RESULTS
    nc = _get_nc()
    in_maps = make_in_maps(xq, xkv, Q, K, V, O)
    res = run_bass_kernel_spmd(nc, in_maps, core_ids=list(range(8)), **RUN_KWARGS)
    LAST_RESULTS = res
    outs = [np.asarray(res.results[c]["out"], np.float32) for c in range(8)]
    return np.stack([outs[2 * b] + outs[2 * b + 1] for b in range(B)], axis=0)


# revision 13
# speedup vs baseline: 1.1137x; 1.0279x over previous
"""Multi-head attention kernel for 8 TRN2 NeuronCores.

Reference: out = einsum('dha,blha->bld', O, softmax(q k^T) v) with
q/k/v = einsum('dha,bld->blha', W, x);  B=4, L=2048, D=1024, H=16, A=64.

Sharding: core c handles batch b = c//2 and head-group hg = c%2 (8 heads).
Each core computes a partial output [L, D] summed over its 8 heads; the host
adds the two head-group partials per batch.

Per-core layout (all "T" = transposed so contractions sit on SBUF partitions):
  phase 1: qT/kT/vT = W^T @ xT via fp32r matmuls (x streamed fp32r from DRAM,
           head pairs packed to M=128, weights stationary reused across two
           lq tiles); v then PE-transposed to natural [Lk, A] bf16 with a
           ones column (softmax denominators come free in the ctx matmul).
  phase 2: per head pair, interleaved even/odd so the K=64 scores matmuls
           land in disjoint PE row groups (tile_position row packing):
           scoresT[lk,lq] = kT^T qT (fp32r); exp on ACT psum->sbuf bf16
           (no max subtraction: |scores| < ~60 so fp32 exp is safe);
           ctx_aug[65,lq] accumulates v_aug^T @ expT over 16 lk chunks;
           normalize = reciprocal_approx_fast on the denominator row +
           DRAM-bounce partition broadcast + DVE multiply -> ctxT pair tile
           [128, lq] (odd head placed via SBUF->SBUF DMA); output projection
           K=128 over pair tiles, heads summed in PSUM -> DMA fp32 out.
"""

import sys

sys.path.insert(0, "/opt/trn_rl_repo")

from contextlib import ExitStack

import numpy as np
import ml_dtypes

import concourse.bass as bass  # noqa: F401
import concourse.tile as tile
from concourse import bacc, mybir
from concourse.bass_utils import run_bass_kernel_spmd
from concourse.masks import make_identity

B, L, D, H, A = 4, 2048, 1024, 16, 64
HC = 8          # heads per core
NP = HC // 2    # head pairs per core
DC = D // 128   # d chunks
LC = L // 128   # l chunks

f32 = mybir.dt.float32
bf16 = mybir.dt.bfloat16
f32r = mybir.dt.float32r
ExpF = mybir.ActivationFunctionType.Exp


def build_graph():
    nc = bacc.Bacc("TRN2", target_bir_lowering=False, debug=False, num_devices=8)
    xqT_e = nc.dram_tensor("xqT", [D, L], f32r, kind="ExternalInput").ap()
    xkvT_e = nc.dram_tensor("xkvT", [D, L], f32r, kind="ExternalInput").ap()
    Qw_e = nc.dram_tensor("Qw", [D, HC * A], f32r, kind="ExternalInput").ap()
    Kw_e = nc.dram_tensor("Kw", [D, HC * A], f32r, kind="ExternalInput").ap()
    Vw_e = nc.dram_tensor("Vw", [D, HC * A], f32r, kind="ExternalInput").ap()
    OwT_e = nc.dram_tensor("OwT", [HC * A, D], bf16, kind="ExternalInput").ap()
    out_e = nc.dram_tensor("out", [L, D], f32, kind="ExternalOutput").ap()

    with tile.TileContext(nc) as tc, ExitStack() as ctx:
        pers = ctx.enter_context(tc.tile_pool(name="pers", bufs=1))
        qT = [pers.tile([128, L], bf16, tag=f"qT{p}", name=f"qT{p}") for p in range(NP)]
        kT = [pers.tile([128, L], bf16, tag=f"kT{p}", name=f"kT{p}") for p in range(NP)]
        # v_aug[h]: [lk chunk part, chunk, 0:64 v | 64 ones | 65 pad]
        vaug = [
            pers.tile([128, LC, 66], bf16, tag=f"vaug{h}", name=f"vaug{h}")
            for h in range(HC)
        ]
        ident = pers.tile([128, 128], bf16, tag="ident", name="ident")
        make_identity(nc, ident[:])
        for h in range(HC):
            nc.vector.memset(vaug[h][:, :, 64:65], 1.0)

        # ---------------- phase 1: projections ----------------
        with tc.tile_pool(name="wp", bufs=1) as wp, \
             tc.tile_pool(name="xin", bufs=4) as xp, \
             tc.tile_pool(name="vtp", bufs=1) as vtp, \
             tc.tile_pool(name="pp1", bufs=8, space="PSUM") as pp1:
            Qc = [wp.tile([128, HC * A], f32r, tag=f"Qc{d}", name=f"Qc{d}") for d in range(DC)]
            Kc = [wp.tile([128, HC * A], f32r, tag=f"Kc{d}", name=f"Kc{d}") for d in range(DC)]
            Vc = [wp.tile([128, HC * A], f32r, tag=f"Vc{d}", name=f"Vc{d}") for d in range(DC)]
            for d in range(DC):
                nc.sync.dma_start(out=Qc[d][:], in_=Qw_e[d * 128:(d + 1) * 128, :])
                nc.sync.dma_start(out=Kc[d][:], in_=Kw_e[d * 128:(d + 1) * 128, :])
                nc.sync.dma_start(out=Vc[d][:], in_=Vw_e[d * 128:(d + 1) * 128, :])
            vT = [vtp.tile([128, L], bf16, tag=f"vT{p}", name=f"vT{p}") for p in range(NP)]

            # one pass per projection; weights stationary reused across the
            # two 512-wide lq tiles of each half, 8 psum accumulators live
            def proj_pass(which, x_e, Wc, emit_out):
                for lqh in range(2):
                    ps = [
                        [
                            pp1.tile([128, 512], f32, tag="qk", bufs=8,
                                     name=f"ps_{which}_{lqh}_{p}_{j}")
                            for j in range(2)
                        ]
                        for p in range(NP)
                    ]
                    for d in range(DC):
                        xt = xp.tile([128, 1024], f32r, tag="x", bufs=4,
                                     name=f"x_{which}_{lqh}_{d}")
                        lo = lqh * 1024
                        nc.sync.dma_start(
                            out=xt[:],
                            in_=x_e[d * 128:(d + 1) * 128, lo:lo + 1024])
                        for p in range(NP):
                            for j in range(2):
                                nc.tensor.matmul(
                                    ps[p][j][:],
                                    lhsT=Wc[d][:, p * 128:(p + 1) * 128],
                                    rhs=xt[:, j * 512:(j + 1) * 512],
                                    start=(d == 0), stop=(d == DC - 1))
                    for p in range(NP):
                        for j in range(2):
                            emit_out(p, lqh * 2 + j, ps[p][j])

            proj_pass("q", xqT_e, Qc,
                      lambda p, lq, pst: nc.vector.tensor_copy(
                          qT[p][:, lq * 512:(lq + 1) * 512], pst[:]))

            # merged k+v pass: one xkv stream feeds both projections
            # (8 psum accumulators live: 4 k + 4 v)
            for lq in range(4):
                pk = [pp1.tile([128, 512], f32, tag="qk", bufs=8,
                               name=f"pk{lq}_{p}") for p in range(NP)]
                pv = [pp1.tile([128, 512], f32, tag="qk", bufs=8,
                               name=f"pv{lq}_{p}") for p in range(NP)]
                for d in range(DC):
                    xt = xp.tile([128, 512], f32r, tag="xkv", bufs=6,
                                 name=f"xkv_{lq}_{d}")
                    nc.sync.dma_start(
                        out=xt[:],
                        in_=xkvT_e[d * 128:(d + 1) * 128, lq * 512:(lq + 1) * 512])
                    for p in range(NP):
                        nc.tensor.matmul(
                            pk[p][:], lhsT=Kc[d][:, p * 128:(p + 1) * 128],
                            rhs=xt[:], start=(d == 0), stop=(d == DC - 1))
                    for p in range(NP):
                        nc.tensor.matmul(
                            pv[p][:], lhsT=Vc[d][:, p * 128:(p + 1) * 128],
                            rhs=xt[:], start=(d == 0), stop=(d == DC - 1))
                for p in range(NP):
                    nc.vector.tensor_copy(kT[p][:, lq * 512:(lq + 1) * 512], pk[p][:])
                    nc.vector.tensor_copy(vT[p][:, lq * 512:(lq + 1) * 512], pv[p][:])

            # v transposes: vT [2h*a, lk] -> v natural [lk, a] per head
            for p in range(NP):
                for c in range(LC):
                    pt = pp1.tile([128, 128], bf16, tag="qk", bufs=8, name=f"pt{p}_{c}")
                    nc.tensor.transpose(pt[:], vT[p][:, c * 128:(c + 1) * 128], ident[:])
                    nc.vector.tensor_copy(vaug[2 * p][:, c, 0:64], pt[:, 0:64])
                    nc.vector.tensor_copy(vaug[2 * p + 1][:, c, 0:64], pt[:, 64:128])

        # ---------------- phase 2: attention + output projection ----------------
        with tc.tile_pool(name="owp", bufs=1) as owp, \
             tc.tile_pool(name="p2p", bufs=1) as p2p, \
             tc.tile_pool(name="drp", bufs=4, space="DRAM") as drp, \
             tc.tile_pool(name="expp", bufs=8) as ep, \
             tc.tile_pool(name="pp2", bufs=1, space="PSUM") as pp2:
            # O weights pair-stacked: chunk c rows = (head 2c | head 2c+1) x a
            ow = [owp.tile([128, D], bf16, tag=f"ow{c}", name=f"ow{c}") for c in range(NP)]
            for c in range(NP):
                nc.sync.dma_start(out=ow[c][:], in_=OwT_e[c * 128:(c + 1) * 128, :])

            for strip in range(2):
                ctxp = []
                for p in range(NP):
                    ctp = p2p.tile([128, 1024], bf16, tag="ctxT", bufs=6,
                                   name=f"ctp{strip}_{p}")
                    pcs = [
                        [
                            pp2.tile([65, 512], f32, tag="c", bufs=4,
                                     name=f"pc{strip}_{p}_{h2}_{s}")
                            for s in range(2)
                        ]
                        for h2 in range(2)
                    ]
                    for c in range(LC):
                        sts = [
                            pp2.tile([128, 1024], f32, tag="s", bufs=2,
                                     name=f"st{strip}_{p}_{h2}_{c}")
                            for h2 in range(2)
                        ]
                        # h2-outer: consecutive matmuls reuse the stationary k
                        # chunk; the next head's LDWEIGHTS (disjoint row group,
                        # bases 0/64) pulls ahead of the in-flight matmuls
                        for h2 in range(2):
                            base = 64 * h2
                            for sub in range(2):
                                lo = strip * 1024 + sub * 512
                                nc.tensor.matmul(
                                    sts[h2][:, sub * 512:(sub + 1) * 512],
                                    lhsT=kT[p][base:base + 64, c * 128:(c + 1) * 128],
                                    rhs=qT[p][base:base + 64, lo:lo + 512],
                                    start=True, stop=True)
                        for h2 in range(2):
                            et = ep.tile([128, 1024], bf16, tag="exp",
                                         name=f"et{strip}_{p}_{h2}_{c}")
                            nc.scalar.activation(et[:], sts[h2][:], ExpF)
                            for sub in range(2):
                                nc.tensor.matmul(
                                    pcs[h2][sub][:],
                                    lhsT=vaug[2 * p + h2][:, c, 0:65],
                                    rhs=et[:, sub * 512:(sub + 1) * 512],
                                    start=(c == 0), stop=(c == LC - 1))
                    for h2 in range(2):
                        cto = None
                        if h2 == 1:
                            cto = p2p.tile([64, 1024], bf16, tag="cto", bufs=3,
                                           name=f"cto{strip}_{p}")
                        for sub in range(2):
                            rt = p2p.tile([65, 512], f32, tag="recip", bufs=2,
                                          name=f"rt{strip}_{p}_{h2}_{sub}")
                            nc.vector.reciprocal(
                                rt[64:65, :], pcs[h2][sub][64:65, :])
                            dr = drp.tile([1, 512], f32, tag="dr", bufs=4,
                                          name=f"dr{strip}_{p}_{h2}_{sub}")
                            nc.sync.dma_start(out=dr[:], in_=rt[64:65, :])
                            pbs = p2p.tile([64, 512], f32, tag="bcast", bufs=2,
                                           name=f"pbs{strip}_{p}_{h2}_{sub}")
                            dr_bcast = bass.AP(
                                tensor=dr[:].tensor,
                                offset=dr[:].offset,
                                ap=[[0, 64], [1, 512]],
                            )
                            nc.sync.dma_start(out=pbs[:], in_=dr_bcast)
                            dst = (ctp[0:64, sub * 512:(sub + 1) * 512]
                                   if h2 == 0 else
                                   cto[:, sub * 512:(sub + 1) * 512])
                            nc.vector.tensor_mul(
                                dst, pcs[h2][sub][0:64, :], pbs[:])
                        if h2 == 1:
                            # odd head into pair-tile partitions 64..127
                            nc.sync.dma_start(out=ctp[64:128, :], in_=cto[:])
                    ctxp.append(ctp)

                # output projection for this strip: K=128 per pair, pairs
                # accumulated in PSUM
                for lqs in range(8):
                    for dt_ in range(2):
                        po = pp2.tile([128, 512], f32, tag="c", bufs=4,
                                      name=f"po{strip}_{lqs}_{dt_}")
                        for p in range(NP):
                            nc.tensor.matmul(
                                po[:],
                                lhsT=ctxp[p][:, lqs * 128:(lqs + 1) * 128],
                                rhs=ow[p][:, dt_ * 512:(dt_ + 1) * 512],
                                start=(p == 0), stop=(p == NP - 1))
                        ost = p2p.tile([128, 512], f32, tag="ost", bufs=3,
                                       name=f"ost{strip}_{lqs}_{dt_}")
                        nc.vector.tensor_copy(ost[:], po[:])
                        row = strip * 1024 + lqs * 128
                        nc.sync.dma_start(
                            out=out_e[row:row + 128, dt_ * 512:(dt_ + 1) * 512],
                            in_=ost[:])

    nc.compile()
    return nc


_NC = None


def _get_nc():
    global _NC
    if _NC is None:
        _NC = build_graph()
    return _NC


# test harness can override, e.g. {"trace": True}
RUN_KWARGS: dict = {}
LAST_RESULTS = None


def make_in_maps(xq, xkv, Q, K, V, O):
    xq = np.asarray(xq, np.float32)
    xkv = np.asarray(xkv, np.float32)
    Q = np.asarray(Q, np.float32)
    K = np.asarray(K, np.float32)
    V = np.asarray(V, np.float32)
    O = np.asarray(O, np.float32)
    in_maps = []
    for core in range(8):
        b, hg = divmod(core, 2)
        hs = slice(hg * HC, (hg + 1) * HC)
        in_maps.append({
            "xqT": np.ascontiguousarray(xq[b].T),
            "xkvT": np.ascontiguousarray(xkv[b].T),
            "Qw": np.ascontiguousarray(Q[:, hs, :].reshape(D, HC * A)),
            "Kw": np.ascontiguousarray(K[:, hs, :].reshape(D, HC * A)),
            "Vw": np.ascontiguousarray(V[:, hs, :].reshape(D, HC * A)),
            "OwT": np.ascontiguousarray(
                O[:, hs, :].reshape(D, HC * A).T).astype(ml_dtypes.bfloat16),
        })
    return in_maps


def kernel(xq, xkv, Q, K, V, O):
    global LAST_RESULTS
    nc = _get_nc()
    in_maps = make_in_maps(xq, xkv, Q, K, V, O)
    res = run_bass_kernel_spmd(nc, in_maps, core_ids=list(range(8)), **RUN_KWARGS)
    LAST_RESULTS = res
    outs = [np.asarray(res.results[c]["out"], np.float32) for c in range(8)]
    return np.stack([outs[2 * b] + outs[2 * b + 1] for b in range(B)], axis=0)
